# revision 29
# baseline (speedup 1.0000x reference)
"""Multi-head attention (B=4, T=2048, D=1024, H=16) on 8 trn2 NeuronCores.

Sharding: core c -> batch b = c>>1, head-group g = c&1 (8 heads each).
Each core computes its 8 heads' attention + a partial output projection
(rows of Wo for its heads); host sums the two partials per batch and adds bo.

All matmuls run in float32r (~13-bit mantissa, full PE rate); softmax is
unnormalized-exp (scores are O(1), no max subtraction needed) with the row
sum l computed by an ones-matmul packed into spare PE column groups.
"""
import sys

sys.path.insert(0, '/opt/trn_rl_repo')

import numpy as np

import concourse.bass as bass
import concourse.mybir as mybir
import concourse.tile as tile
from concourse import bacc
from concourse.bass_utils import run_bass_kernel_spmd
from concourse.masks import make_identity

F32 = mybir.dt.float32
F32R = mybir.dt.float32r

B, T, D = 4, 2048, 1024
H, DH = 16, 64
HPC = 8          # heads per core
NPAIR = HPC // 2  # head pairs per core (2 heads of 64 dims -> 128 partitions)
NTB = T // 512    # 512-wide t blocks
NTC = T // 128    # 128-wide t chunks
NDC = D // 128    # 128-wide D chunks
QC = 1024         # q-range processed per attention inner block
NQC = T // QC
NKC = T // 128    # k chunks (128-wide) per attention block


def build_kernel():
    nc = bacc.Bacc("TRN2", target_bir_lowering=False, debug=False)

    # activations arrive host-transposed: [D, T]
    xq = nc.dram_tensor("xq", [D, T], F32, kind="ExternalInput")
    xk = nc.dram_tensor("xk", [D, T], F32, kind="ExternalInput")
    xv = nc.dram_tensor("xv", [D, T], F32, kind="ExternalInput")
    wq = nc.dram_tensor("wq", [D, 512], F32, kind="ExternalInput")
    wk = nc.dram_tensor("wk", [D, 512], F32, kind="ExternalInput")
    wv = nc.dram_tensor("wv", [D, 512], F32, kind="ExternalInput")
    wo = nc.dram_tensor("wo", [512, D], F32, kind="ExternalInput")
    bq = nc.dram_tensor("bq", [128, 4], F32, kind="ExternalInput")
    bk = nc.dram_tensor("bk", [128, 4], F32, kind="ExternalInput")
    bv = nc.dram_tensor("bv", [128, 512], F32, kind="ExternalInput")
    out = nc.dram_tensor("out", [T, D], F32, kind="ExternalOutput")

    with tile.TileContext(nc) as tc:
        with tc.tile_pool(name="persist", bufs=1) as pp:
            qht = pp.tile([128, NPAIR * T], F32R)   # [dh-pair, (pair, t)]
            kht = pp.tile([128, NPAIR * T], F32R)
            # [t-chunk rows, (tchunk, head, 65)]: per head 64 V dims + ones col
            vh = pp.tile([128, NTC * 520], F32R)
            ot = pp.tile([128, NPAIR * T], F32R)    # [dh-pair, (pair, t)] normalized O^T
            ones_f = pp.tile([128, 1], F32)
            nc.vector.memset(ones_f[:], 1.0)
            # fill all 16*8 per-(tchunk, head) ones columns of vh in one copy
            vh_ones = vh[:].rearrange("p (tc h c) -> p (tc h) c", tc=NTC, h=8)[:, :, 64:65]
            nc.vector.tensor_copy(vh_ones, ones_f[:].to_broadcast((128, NTC * 8, 1)))
            bq_sb = pp.tile([128, 4], F32)
            bk_sb = pp.tile([128, 4], F32)
            bv_sb = pp.tile([128, 512], F32)

            # ------- Phase 1: load pre-transposed activations + projections -------
            for xdram, wdram, kind in ((xq, wq, "q"), (xk, wk, "k"), (xv, wv, "v")):
                with tc.tile_pool(name=f"p1_{kind}", bufs=1) as wp, \
                     tc.tile_pool(name=f"p1s_{kind}", bufs=4) as sp, \
                     tc.tile_pool(name=f"p1x_{kind}", bufs=8) as xp, \
                     tc.tile_pool(name=f"p1t_{kind}", bufs=2) as tp, \
                     tc.tile_pool(name=f"ps2_{kind}", bufs=6, space="PSUM") as prp:
                    # interleave tb0 activation chunks with W chunks so the
                    # first projection matmul has both operands ASAP
                    w_sb = wp.tile([128, NDC * 512], F32R, name=f"w_{kind}")
                    xin0 = []
                    for d in range(NDC):
                        xi = xp.tile([128, 512], F32, name="xstg", tag="xstg")
                        nc.sync.dma_start(
                            xi[:], xdram.ap()[d * 128:(d + 1) * 128, 0:512])
                        xin0.append(xi)
                        stg = sp.tile([128, 512], F32, name="wstg", tag="wstg")
                        nc.sync.dma_start(stg[:], wdram.ap()[d * 128:(d + 1) * 128, :])
                        if d % 2 == 0:
                            nc.scalar.copy(w_sb[:, d * 512:(d + 1) * 512], stg[:])
                        else:
                            nc.vector.tensor_copy(w_sb[:, d * 512:(d + 1) * 512], stg[:])
                        if kind == "q" and d == 0:
                            # biases are tiny; keep them off the startup DMA path
                            nc.sync.dma_start(bq_sb[:], bq.ap())
                            nc.sync.dma_start(bk_sb[:], bk.ap())
                            nc.sync.dma_start(bv_sb[:], bv.ap())

                    for tb in range(NTB):  # t blocks of 512
                        xt = tp.tile([128, NDC * 512], F32R, name="xt", tag="xt")
                        for d in range(NDC):
                            if tb == 0:
                                xi = xin0[d]
                            else:
                                xi = xp.tile([128, 512], F32, name="xstg", tag="xstg")
                                nc.sync.dma_start(
                                    xi[:],
                                    xdram.ap()[d * 128:(d + 1) * 128,
                                               tb * 512:(tb + 1) * 512])
                            if d % 2 == 0:
                                nc.scalar.copy(xt[:, d * 512:(d + 1) * 512], xi[:])
                            else:
                                nc.vector.tensor_copy(xt[:, d * 512:(d + 1) * 512], xi[:])

                        if kind in ("q", "k"):
                            dst = qht if kind == "q" else kht
                            bias = bq_sb if kind == "q" else bk_sb
                            for p in range(NPAIR):
                                pj = prp.tile([128, 512], F32, name="pj", tag="pj")
                                for d in range(NDC):
                                    nc.tensor.matmul(
                                        pj[:],
                                        w_sb[:, d * 512 + p * 128: d * 512 + (p + 1) * 128],
                                        xt[:, d * 512:(d + 1) * 512],
                                        start=(d == 0), stop=(d == NDC - 1))
                                nc.vector.tensor_scalar_add(
                                    dst[:, p * T + tb * 512: p * T + (tb + 1) * 512],
                                    pj[:], bias[:, p:p + 1])
                        else:
                            for ts in range(4):
                                pj = prp.tile([128, 512], F32, name="pjv", tag="pj")
                                for d in range(NDC):
                                    nc.tensor.matmul(
                                        pj[:],
                                        xt[:, d * 512 + ts * 128: d * 512 + (ts + 1) * 128],
                                        w_sb[:, d * 512:(d + 1) * 512],
                                        start=(d == 0), stop=(d == NDC - 1))
                                tc16 = tb * 4 + ts
                                dst = vh[:, tc16 * 520:(tc16 + 1) * 520].rearrange(
                                    "p (h c) -> p h c", h=8)[:, :, 0:64]
                                nc.vector.tensor_tensor(
                                    dst,
                                    pj[:].rearrange("p (h c) -> p h c", h=8),
                                    bv_sb[:].rearrange("p (h c) -> p h c", h=8),
                                    op=mybir.AluOpType.add)

            # ---------------- Phase 2: attention ----------------
            with tc.tile_pool(name="p3w", bufs=1) as wp3, \
                 tc.tile_pool(name="p3s", bufs=2) as sp3:
                wo_sb = wp3.tile([128, 4 * D], F32R)
                wo_emitted = False
                attn_psum = tc.tile_pool(name="ps_s", bufs=1, space="PSUM")
                spsum = attn_psum.__enter__()
                attn_psum2 = tc.tile_pool(name="ps_o", bufs=2, space="PSUM")
                opsum = attn_psum2.__enter__()
                ptp_cm = tc.tile_pool(name="p2pt", bufs=4)
                ptp = ptp_cm.__enter__()
                rp_cm = tc.tile_pool(name="p2r", bufs=2)
                rp = rp_cm.__enter__()
                def emit_pv(blk, kc, pt, h):
                    p_, _, ops_ = blk
                    vcol = kc * 520 + (2 * p_ + h) * 65
                    for n in range(QC // 512):
                        rhs = pt[:, h * QC + n * 512: h * QC + (n + 1) * 512]
                        nc.tensor.matmul(
                            ops_[h][:, n * 512:(n + 1) * 512],
                            vh[:, vcol:vcol + 65], rhs,
                            start=(kc == 0), stop=(kc == NKC - 1))

                def emit_norm(blk):
                    p_, qc_, ops_ = blk
                    q0_ = p_ * T + qc_ * QC
                    for h in range(2):
                        r1 = rp.tile([1, QC], F32, name="r1", tag="r1")
                        nc.vector.reciprocal(r1[:], ops_[h][64:65, :])
                        r64 = rp.tile([64, QC], F32, name="r64", tag="r64")
                        nc.gpsimd.partition_broadcast(r64[:], r1[:])
                        nc.vector.tensor_tensor(
                            ot[h * 64:(h + 1) * 64, q0_:q0_ + QC],
                            ops_[h][0:64, :], r64[:], op=mybir.AluOpType.mult)

                # PV lags S/exp by one k-chunk, carried across block
                # boundaries so neither PE nor ACT drain between blocks
                prev = None   # (blk, kc, pt)
                for p in range(NPAIR):
                    for qc in range(NQC):
                        if p == 0 and qc == 1 and not wo_emitted:
                            # prefetch + round Wo on idle DMA/DVE during attention
                            wo_emitted = True
                            for hc in range(4):
                                stg = sp3.tile([128, D], F32, name="wostg", tag="wostg")
                                nc.sync.dma_start(
                                    stg[:], wo.ap()[hc * 128:(hc + 1) * 128, :])
                                nc.vector.tensor_copy(
                                    wo_sb[:, hc * D:(hc + 1) * D], stg[:])
                        q0 = p * T + qc * QC
                        ops = [opsum.tile([65, QC], F32, name=f"ops{h}", tag="ops")
                               for h in range(2)]
                        blk = (p, qc, ops)
                        for kc in range(NKC):
                            k0 = p * T + kc * 128
                            sps = [spsum.tile([128, QC], F32, name=f"sps{h}",
                                              tag=f"sps{h}") for h in range(2)]
                            pt = ptp.tile([128, 2048], F32R, name="pt", tag="pt")
                            for h in range(2):
                                hp = h * 64
                                for n in range(QC // 512):
                                    nc.tensor.matmul(
                                        sps[h][:, n * 512:(n + 1) * 512],
                                        kht[hp:hp + 64, k0:k0 + 128],
                                        qht[hp:hp + 64, q0 + n * 512: q0 + (n + 1) * 512],
                                        start=True, stop=True,
                                        tile_position=(hp, 0))
                                nc.scalar.activation(
                                    pt[:, h * QC:(h + 1) * QC], sps[h][:],
                                    mybir.ActivationFunctionType.Exp, scale=0.125)
                                if prev is not None:
                                    emit_pv(prev[0], prev[1], prev[2], h)
                            if prev is not None and prev[1] == NKC - 1:
                                # previous block's accumulation just closed
                                emit_norm(prev[0])
                            prev = (blk, kc, pt)
                # epilogue: final block's last PV + norm
                emit_pv(prev[0], prev[1], prev[2], 0)
                emit_pv(prev[0], prev[1], prev[2], 1)
                emit_norm(prev[0])
                rp_cm.__exit__(None, None, None)
                ptp_cm.__exit__(None, None, None)
                attn_psum2.__exit__(None, None, None)
                attn_psum.__exit__(None, None, None)

                # ------------- Phase 3: output projection -------------
                with tc.tile_pool(name="p3o", bufs=3) as op3, \
                     tc.tile_pool(name="ps3", bufs=4, space="PSUM") as prp3:
                    for tc16 in range(NTC):
                        ostg = op3.tile([128, D], F32, name="ostg", tag="ostg")
                        for nn in range(2):
                            pj = prp3.tile([128, 512], F32, name="pj3", tag="pj3")
                            for p in range(NPAIR):
                                nc.tensor.matmul(
                                    pj[:],
                                    ot[:, p * T + tc16 * 128: p * T + (tc16 + 1) * 128],
                                    wo_sb[:, p * D + nn * 512: p * D + (nn + 1) * 512],
                                    start=(p == 0), stop=(p == NPAIR - 1))
                            if nn == 0:
                                nc.scalar.copy(ostg[:, nn * 512:(nn + 1) * 512], pj[:])
                            else:
                                nc.vector.tensor_copy(ostg[:, nn * 512:(nn + 1) * 512], pj[:])
                            nc.sync.dma_start(
                                out.ap()[tc16 * 128:(tc16 + 1) * 128,
                                         nn * 512:(nn + 1) * 512],
                                ostg[:, nn * 512:(nn + 1) * 512])

    nc.compile()
    return nc


_NC = None


def _get_nc():
    global _NC
    if _NC is None:
        _NC = build_kernel()
    return _NC


def make_in_maps(q, k, v, Wq, bq, Wk, bk, Wv, bv, Wo, bo):
    in_maps = []
    for c in range(8):
        b, g = c >> 1, c & 1
        cs = slice(g * 512, (g + 1) * 512)
        in_maps.append({
            "xq": np.ascontiguousarray(np.asarray(q[b]).T),
            "xk": np.ascontiguousarray(np.asarray(k[b]).T),
            "xv": np.ascontiguousarray(np.asarray(v[b]).T),
            "wq": np.ascontiguousarray(Wq[:, cs]),
            "wk": np.ascontiguousarray(Wk[:, cs]),
            "wv": np.ascontiguousarray(Wv[:, cs]),
            "wo": np.ascontiguousarray(Wo[cs, :]),
            "bq": np.ascontiguousarray(bq[cs].reshape(4, 128).T),
            "bk": np.ascontiguousarray(bk[cs].reshape(4, 128).T),
            "bv": np.ascontiguousarray(np.broadcast_to(bv[cs], (128, 512))),
        })
    return in_maps


def kernel(q, k, v, Wq, bq, Wk, bk, Wv, bv, Wo, bo):
    nc = _get_nc()
    in_maps = make_in_maps(q, k, v, Wq, bq, Wk, bk, Wv, bv, Wo, bo)
    res = run_bass_kernel_spmd(nc, in_maps, core_ids=list(range(8)))
    out = np.zeros((B, T, D), np.float32)
    for c in range(8):
        out[c >> 1] += res.results[c]["out"]
    out += bo.astype(np.float32)
    return out
